# revision 1
# baseline (speedup 1.0000x reference)
"""Trainium2 Bass kernel for nn_BiLSTM_57440892617018.

2-layer bidirectional LSTM (independent fw / bw stacks, merge_mode='ave'),
B=2048, T=200, D=U=128. Data-parallel over batch across 8 NeuronCores.

Per-core structure: ONE software-pipelined loop with 4 recurrent streams,
parity-staggered: even supersteps advance the forward pair (L0f, L1f — layer
1 lagging layer 0 by one timestep), odd supersteps advance the backward pair
on the time-reversed sequence. The stagger gives every stream's recurrence
chain two superstep periods of latency budget, so pair-merged ACT
instructions (sigmoid over [i0|i1|f0|f1|o0|o1], tanh over [g0|g1], tanh over
[c0|c1]) stay off the tight chain. The tail of each superstep (tanh(c),
h-mul, merge, store) is emitted one block later so the ACT queue interleaves
the two parities with no idle window. All matmuls fp16 on PE (fp32 PSUM
accumulation), elementwise fp16 on DVE at 2x mode. Layer-1 outputs are
staged in SBUF (half of each direction) and merged on PE via 0.5*I matmuls,
which also yields the [b, d] layout the output DMA needs.
"""

import numpy as np
import ml_dtypes

import concourse.bass as bass
import concourse.tile as tile
from concourse import bacc, mybir
from concourse.bass_utils import run_bass_kernel_spmd

F32 = mybir.dt.float32
DT2 = mybir.dt.float16
DT2_NP = np.float16
AF = mybir.ActivationFunctionType

B, T, D, U = 2048, 200, 128, 128
NCORES = 8
BL = B // NCORES          # 256 batch per core
NB = BL // 128            # 2 b-tiles
BCOL = 128 * NB           # 256 free columns (batch)

# gate order inside the reference 4u axis: i, f, g, o
GATE_COLS = {"i": 0, "f": 128, "g": 256, "o": 384}
SLOTS = ["Wi", "Wf", "Wo", "Wg", "Ui", "Uf", "Uo", "Ug"]

_cache = {}


def _wcol(di, l, slot):
    return ((di * 2 + l) * 8 + slot) * 128


def _build(uniform_bias, bias_val, Tn=T, reps=1, loop_R=0, tiny_x=False,
           split_o=True):
    nc = bacc.Bacc("TRN2", target_bir_lowering=False, debug=False,
                   num_devices=NCORES)
    Th = Tn // 2

    xs = nc.dram_tensor("xs", [BL, 1 if tiny_x else Tn, D], F32,
                        kind="ExternalInput")
    wts = nc.dram_tensor("wts", [2, 2, 8, 128, 128], DT2, kind="ExternalInput")
    ident2 = nc.dram_tensor("ident2", [128, 256], DT2, kind="ExternalInput")
    biases = nc.dram_tensor("biases", [128, 16], F32, kind="ExternalInput")
    if loop_R:
        dummyout = nc.dram_tensor("dummyout", [128, 4], F32,
                                  kind="ExternalOutput")
    else:
        out = nc.dram_tensor("out", [BL, Tn, D], F32, kind="ExternalOutput")

    NS = 2 * Tn + 2   # supersteps

    with tile.TileContext(nc) as tc:
        with (
            tc.tile_pool(name="wpool", bufs=1) as wpool,
            tc.tile_pool(name="stage", bufs=1) as stage,
            tc.tile_pool(name="xraw", bufs=6) as xraw,
            tc.tile_pool(name="xtp", bufs=4) as xtp,
            tc.tile_pool(name="hpool", bufs=6) as hpool,
            tc.tile_pool(name="cpool", bufs=1) as cpool,
            tc.tile_pool(name="gsb", bufs=3) as gsb,
            tc.tile_pool(name="tcp", bufs=4) as tcp,
            tc.tile_pool(name="tmp", bufs=8) as tmp,
            tc.tile_pool(name="outp", bufs=3) as outp,
            tc.tile_pool(name="dramp", bufs=1, space="DRAM") as dramp,
            tc.tile_pool(name="psig", bufs=2, space="PSUM") as psig,
            tc.tile_pool(name="pg", bufs=1, space="PSUM") as pg,
            tc.tile_pool(name="pscr", bufs=1, space="PSUM") as pscr,
        ):
            # ---- constants / weights ----
            wslab = wpool.tile([128, 4096], DT2)
            for di in range(2):
                for l in range(2):
                    base = _wcol(di, l, 0)
                    nc.sync.dma_start(
                        wslab[:, base:base + 1024].rearrange(
                            "k (s m) -> k s m", s=8),
                        wts.ap()[di, l].rearrange("s k m -> k s m"))
            idt = wpool.tile([128, 256], DT2)
            nc.sync.dma_start(idt[:], ident2.ap())
            eye = idt[:, 0:128]
            half_eye = idt[:, 128:256]
            bsb = wpool.tile([128, 16], F32)
            nc.sync.dma_start(bsb[:], biases.ap())

            if loop_R:
                out_int = dramp.tile([BL, Tn, D], F32)

                def out_ap():
                    return out_int[:]
            else:
                def out_ap():
                    return out.ap()

            z0 = wpool.tile([128, BCOL], DT2)
            nc.gpsimd.memset(z0[:], 0.0)

            # persistent cell states per direction: [c0|c1] pair tiles
            c_pair = {}
            for di in range(2):
                ct = cpool.tile([128, 2 * BCOL], DT2, tag=f"c{di}")
                nc.gpsimd.memset(ct[:], 0.0)
                c_pair[di] = ct

            # SBUF staging for layer-1 outputs awaiting merge
            stage_f = stage.tile([128, Th * BCOL], DT2, tag="stf")
            stage_b = stage.tile([128, Th * BCOL], DT2, tag="stb")

            # scratch PSUM bank: f32 cols [0:128)+[128:256) = 2 parity slots
            # for x transposes (fp16 via bitcast); [256:512) = merge psum
            scr = pscr.tile([128, 512], F32)

            h_prev = {(0, 0): z0[:], (0, 1): z0[:],
                      (1, 0): z0[:], (1, 1): z0[:]}
            xT_ready = {}
            xr_ready = {}

            def x_load(s):
                m = s // 2
                if m >= Tn:
                    return
                di = s % 2
                tsrc = 0 if tiny_x else (m if di == 0 else Tn - 1 - m)
                with nc.named_scope("xload"):
                    xr = xraw.tile([128, NB, 128], F32)
                    nc.sync.dma_start(
                        xr[:],
                        xs.ap()[:, tsrc, :].rearrange("(j p) d -> p j d",
                                                      j=NB))
                xr_ready[s] = xr

            def x_prep(s):
                m = s // 2
                if m >= Tn:
                    return
                xr = xr_ready.pop(s)
                sc = nc.named_scope("xprep"); sc.__enter__()
                xb = xtp.tile([128, BCOL], DT2, tag="xb")
                nc.vector.tensor_copy(
                    xb[:], xr[:].rearrange("p j d -> p (j d)"))
                par = (s % 2) * 128
                xpd = scr[:, par:par + 128].bitcast(DT2)   # [128, 256] fp16
                nc.tensor.transpose(xpd[:, 0:128], xb[:, 0:128], eye)
                nc.tensor.transpose(xpd[:, 128:256], xb[:, 128:256], eye)
                xT = xtp.tile([128, BCOL], DT2, tag="xT")
                nc.vector.tensor_copy(xT[:], xpd[:])
                sc.__exit__(None, None, None)
                xT_ready[s] = xT

            def finish_head(pend):
                """Early tail of superstep s-1: tanh(c) and the h products."""
                (di, st0, st1, tt1, gates, lo, hi) = pend
                c_t = c_pair[di]
                tc_t = tcp.tile([128, 2 * BCOL], DT2)
                with nc.named_scope("tanhc"):
                    nc.scalar.activation(tc_t[:, lo:hi], c_t[:, lo:hi],
                                         AF.Tanh)
                merge = None
                if st0:
                    h_t = hpool.tile([128, BCOL], DT2, tag="h0")
                    with nc.named_scope("hmul"):
                        nc.vector.tensor_mul(h_t[:], gates[:, 1024:1280],
                                             tc_t[:, 0:BCOL])
                    h_prev[(di, 0)] = h_t[:]
                if st1:
                    o_ap = gates[:, 1280:1536]
                    stg = stage_f if di == 0 else stage_b
                    stage_this = (tt1 < Th) if di == 0 else (tt1 >= Th)
                    if stage_this:
                        soff = (tt1 if di == 0 else tt1 - Th) * BCOL
                        dst = stg[:, soff:soff + BCOL]
                        nc.vector.tensor_mul(dst, o_ap, tc_t[:, BCOL:2 * BCOL])
                        h_prev[(di, 1)] = dst
                    else:
                        h_t = hpool.tile([128, BCOL], DT2, tag="h1")
                        nc.vector.tensor_mul(h_t[:], o_ap,
                                             tc_t[:, BCOL:2 * BCOL])
                        h_prev[(di, 1)] = h_t[:]
                        merge = (di, tt1, h_t)
                return merge

            def finish_tail(merge):
                """Late tail of superstep s-1: output merge + store."""
                di, tt1, h_t = merge
                sc = nc.named_scope("mergeout"); sc.__enter__()
                ostg = stage_b if di == 0 else stage_f
                ooff = (tt1 - Th if di == 0 else tt1) * BCOL
                other = ostg[:, ooff:ooff + BCOL]
                for j in range(NB):
                    mdst = scr[:, 256 + j * 128:256 + j * 128 + 128]
                    nc.tensor.matmul(
                        mdst, h_t[:, j * 128:j * 128 + 128],
                        half_eye, start=True, stop=False)
                    nc.tensor.matmul(
                        mdst, other[:, j * 128:j * 128 + 128],
                        half_eye, start=False, stop=True)
                ost = outp.tile([128, BCOL], F32)
                nc.vector.tensor_copy(ost[:], scr[:, 256:512])
                nc.sync.dma_start(
                    out_ap()[:, tt1, :].rearrange("(j p) d -> p j d", j=NB),
                    ost[:].rearrange("p (j d) -> p j d", j=NB))
                sc.__exit__(None, None, None)

            rep_loop = True
            pending = None          # ACT/DVE state of superstep s-1
            mm_ctx = {}             # s -> (sig_ps, g_ps) emitted matmuls

            def emit_mms(s):
                """Emit all gate matmuls for superstep s (sig-feeding first)."""
                di = s % 2
                m = s // 2
                st0 = m < Tn
                st1 = 1 <= m <= Tn
                if not (st0 or st1) or s >= NS:
                    return
                sc = nc.named_scope("gatemm"); sc.__enter__()
                sig_ps = psig.tile([128, 1536], F32)
                g_ps = pg.tile([128, 512], F32)
                rhs = {}
                for stream, active in ((0, st0), (1, st1)):
                    if active:
                        rhs[stream] = (
                            xT_ready.pop(s) [:] if stream == 0
                            else h_prev[(di, 0)],
                            h_prev[(di, stream)])
                for gi in (0, 1):     # i, f feed the early sigmoid
                    for stream in rhs:
                        dst = sig_ps[:, gi * 512 + stream * 256:
                                     gi * 512 + stream * 256 + 256]
                        nc.tensor.matmul(
                            dst, wslab[:, _wcol(di, stream, gi):
                                       _wcol(di, stream, gi) + 128],
                            rhs[stream][0], start=True, stop=False)
                        nc.tensor.matmul(
                            dst, wslab[:, _wcol(di, stream, 4 + gi):
                                       _wcol(di, stream, 4 + gi) + 128],
                            rhs[stream][1], start=False, stop=True)
                for stream in rhs:
                    dstg = g_ps[:, stream * 256:stream * 256 + 256]
                    nc.tensor.matmul(
                        dstg, wslab[:, _wcol(di, stream, 3):
                                    _wcol(di, stream, 3) + 128],
                        rhs[stream][0], start=True, stop=False)
                    nc.tensor.matmul(
                        dstg, wslab[:, _wcol(di, stream, 7):
                                    _wcol(di, stream, 7) + 128],
                        rhs[stream][1], start=False, stop=True)
                gi = 2                # o gates: off the critical chain
                for stream in rhs:
                    dst = sig_ps[:, gi * 512 + stream * 256:
                                 gi * 512 + stream * 256 + 256]
                    nc.tensor.matmul(
                        dst, wslab[:, _wcol(di, stream, gi):
                                   _wcol(di, stream, gi) + 128],
                        rhs[stream][0], start=True, stop=False)
                    nc.tensor.matmul(
                        dst, wslab[:, _wcol(di, stream, 4 + gi):
                                   _wcol(di, stream, 4 + gi) + 128],
                        rhs[stream][1], start=False, stop=True)
                sc.__exit__(None, None, None)
                mm_ctx[s] = (sig_ps, g_ps)

            import contextlib
            if loop_R:
                cnt = wpool.tile([128, 4], F32, tag="cnt")
                nc.gpsimd.memset(cnt[:], 0.0)
            loop_cm = tc.For_i(0, loop_R, 1) if loop_R else \
                contextlib.nullcontext()
            with loop_cm:
             if loop_R:
                 nc.vector.tensor_scalar_add(cnt[:], cnt[:], 1.0)
             for rep in range(reps):
              if rep > 0 or loop_R:
                for di_ in range(2):
                    nc.gpsimd.memset(c_pair[di_][:], 0.0)
                h_prev.update({(0, 0): z0[:], (0, 1): z0[:],
                               (1, 0): z0[:], (1, 1): z0[:]})
                pending = None
              for s in range(NS):
                di = s % 2
                m = s // 2
                st0 = m < Tn
                st1 = 1 <= m <= Tn
                t1 = m - 1
                tt1 = t1 if di == 0 else Tn - 1 - t1

                # ---- prologue / x prefetch ----
                if s == 0:
                    for ps in range(4):
                        x_load(ps)
                    x_prep(0)
                    x_prep(1)
                    emit_mms(0)
                if s + 4 < NS:
                    x_load(s + 4)

                # ---- early tail of superstep s-1 (other parity) ----
                merge = None
                if pending is not None:
                    merge = finish_head(pending)
                    pending = None

                # ---- activations for superstep s ----
                sig_ps, g_ps = mm_ctx.pop(s)
                gates = gsb.tile([128, 2048], DT2)
                lo = 0 if st0 else BCOL
                hi = 2 * BCOL if st1 else BCOL
                sig_o = None
                if uniform_bias and st0 and st1:
                    ncol = 1024 if split_o else 1536
                    with nc.named_scope("sig"):
                        nc.scalar.activation(gates[:, 0:ncol],
                                             sig_ps[:, 0:ncol],
                                             AF.Sigmoid, bias=bias_val)
                    with nc.named_scope("tanhg"):
                        nc.scalar.activation(gates[:, 1536:2048], g_ps[:],
                                             AF.Tanh, bias=bias_val)
                    if split_o:
                        sig_o = (gates, sig_ps)
                else:
                    for stream, active in ((0, st0), (1, st1)):
                        if not active:
                            continue
                        l = stream
                        for gi in range(3):
                            cc = gi * 512 + stream * 256
                            bias = (bias_val if uniform_bias else
                                    bsb[:, (di * 2 + l) * 4 + gi:
                                        (di * 2 + l) * 4 + gi + 1])
                            nc.scalar.activation(
                                gates[:, cc:cc + 256],
                                sig_ps[:, cc:cc + 256], AF.Sigmoid, bias=bias)
                        cc = 1536 + stream * 256
                        bias = (bias_val if uniform_bias else
                                bsb[:, (di * 2 + l) * 4 + 3:
                                    (di * 2 + l) * 4 + 4])
                        nc.scalar.activation(
                            gates[:, cc:cc + 256],
                            g_ps[:, stream * 256:stream * 256 + 256],
                            AF.Tanh, bias=bias)

                # ---- DVE combine (pair-merged) ----
                sc = nc.named_scope("combine"); sc.__enter__()
                c_t = c_pair[di]
                tt_t = tmp.tile([128, 2 * BCOL], DT2, tag="tt")
                nc.vector.tensor_mul(tt_t[:, lo:hi],
                                     gates[:, 512 + lo:512 + hi],
                                     c_t[:, lo:hi])
                p_t = tmp.tile([128, 2 * BCOL], DT2, tag="p")
                nc.vector.tensor_mul(p_t[:, lo:hi], gates[:, lo:hi],
                                     gates[:, 1536 + lo:1536 + hi])
                nc.vector.tensor_add(c_t[:, lo:hi], tt_t[:, lo:hi],
                                     p_t[:, lo:hi])
                sc.__exit__(None, None, None)

                # o-gate sigmoid: consumed only by next block's h-mul; fills
                # the ACT gap while DVE finishes the c update
                if sig_o is not None:
                    with nc.named_scope("sigo"):
                        nc.scalar.activation(sig_o[0][:, 1024:1536],
                                             sig_o[1][:, 1024:1536],
                                             AF.Sigmoid, bias=bias_val)

                # ---- late tail of superstep s-1: merge + store ----
                if merge is not None:
                    finish_tail(merge)

                # ---- x pipeline for superstep s+2 ----
                if s + 2 < NS:
                    x_prep(s + 2)

                pending = (di, st0, st1, tt1, gates, lo, hi)

                # ---- gate matmuls for superstep s+1 ----
                emit_mms(s + 1)

              if pending is not None:
                merge = finish_head(pending)
                if merge is not None:
                    finish_tail(merge)
                pending = None
            if loop_R:
                nc.sync.dma_start(dummyout.ap(), cnt[:])
    nc.compile()
    return nc


def _prep_weights(Wf, Uf, Wb, Ub):
    wts = np.zeros((2, 2, 8, 128, 128), dtype=DT2_NP)
    for di, (Wd, Ud) in enumerate(((Wf, Uf), (Wb, Ub))):
        for l in range(2):
            for si, sname in enumerate(SLOTS):
                mat = Wd[l] if sname[0] == "W" else Ud[l]
                g = GATE_COLS[sname[1]]
                wts[di, l, si] = np.asarray(
                    mat[:, g:g + 128], dtype=np.float32).astype(DT2_NP)
    return wts


def _prep_aux(bf, bb):
    eye = np.eye(128, dtype=np.float32)
    ident2 = np.concatenate([eye, 0.5 * eye], axis=1).astype(DT2_NP)
    biases = np.zeros((128, 16), dtype=np.float32)
    for di, bd in enumerate((bf, bb)):
        for l in range(2):
            for gi, gname in enumerate(("i", "f", "o", "g")):
                g = GATE_COLS[gname]
                biases[:, (di * 2 + l) * 4 + gi] = bd[l, g:g + 128]
    return ident2, biases


def kernel(x, Wf, Uf, bf, Wb, Ub, bb):
    x = np.ascontiguousarray(np.asarray(x, dtype=np.float32))
    bf = np.asarray(bf, dtype=np.float32)
    bb = np.asarray(bb, dtype=np.float32)

    bval = float(bf.flat[0])
    uniform = bool(np.all(bf == bval) and np.all(bb == bval))

    key = (uniform, bval if uniform else None)
    if key not in _cache:
        _cache[key] = _build(uniform, bval if uniform else 0.0)
    nc = _cache[key]

    wts = _prep_weights(Wf, Uf, Wb, Ub)
    ident2, biases = _prep_aux(bf, bb)

    in_maps = []
    for c in range(NCORES):
        in_maps.append({
            "xs": x[c * BL:(c + 1) * BL],
            "wts": wts,
            "ident2": ident2,
            "biases": biases,
        })
    res = run_bass_kernel_spmd(nc, in_maps, core_ids=list(range(NCORES)))
    return np.concatenate([res.results[c]["out"] for c in range(NCORES)],
                          axis=0).astype(np.float32)

